# revision 1
# baseline (speedup 1.0000x reference)
"""CharRNN Trainium2 Bass kernel.

Problem: h_t = tanh(W_ax e_{x_t} + W_aa h_{t-1}); out = hs @ Wya^T + b_y.

Strategy:
  * Host folds embedding+input projection into one table  M = emb_table @ Wax^T
    (exact: row-gather commutes with the matmul), so on device the per-step
    input contribution is a dynamic column read M^T[:, idx_t].
  * The 65536-step scan is strictly sequential; every core runs the identical
    scan (replicated - cheapest correct option on this 8-core setup where
    cross-core SBUF p2p is unavailable), with the per-step matvec done on the
    TensorEngine as 64 [128x128] weight-stationary tiles in fp16 (fp32 PSUM
    accumulate).  fp16 weights/state keep the final relative error ~2e-3
    (the tanh dynamics are non-chaotic; errors saturate instead of growing).
  * The output projection is S-sharded: each core projects its own 8192-step
    slice of the h history; the host concatenates the 8 shards.
"""
import os
import sys
import numpy as np

_TRN_REPO = "/opt/trn_rl_repo"
if _TRN_REPO not in sys.path:
    sys.path.insert(0, _TRN_REPO)

SEQ = 65536
HID = 1024
EMB = 512
NCH = 512
NCORES = 8
BLK = 32          # scan steps per loop body
NBODY = SEQ // BLK

_cache = {}


def _build_program():
    import concourse.bacc as bacc
    import concourse.bass as bass
    import concourse.mybir as mybir
    from concourse import tile

    F32 = mybir.dt.float32
    F16 = mybir.dt.float16
    I32 = mybir.dt.int32
    U32 = mybir.dt.uint32
    dt = F16

    NB_CORE = NBODY // NCORES
    nc = bacc.Bacc(detect_race_conditions=False)

    wt = nc.declare_dram_parameter("wt", [128, 64 * 128], dt, isOutput=False)
    mt = nc.declare_dram_parameter("mt", [128, NCH * 8], F32, isOutput=False)
    wy = nc.declare_dram_parameter("wy", [128, 8 * NCH], dt, isOutput=False)
    by = nc.declare_dram_parameter("by", [BLK, NCH], F32, isOutput=False)
    idx = nc.declare_dram_parameter("idx", [1, SEQ], I32, isOutput=False)
    myb = nc.declare_dram_parameter("myb", [1, 1], U32, isOutput=False)
    out = nc.declare_dram_parameter("out", [NB_CORE * BLK, NCH], F32, isOutput=True)
    hlast = nc.declare_dram_parameter("hlast", [128, 8], F32, isOutput=True)
    hst = nc.dram_tensor("hst", [128, 8 * SEQ], dt)

    with tile.TileContext(nc) as tc:
        with (
            tc.tile_pool(name="per", bufs=1) as per,
            tc.tile_pool(name="blkio", bufs=2) as blkio,
            tc.tile_pool(name="psum", bufs=1, space="PSUM") as psumpool,
            tc.tile_pool(name="psum2", bufs=2, space="PSUM") as psumpool2,
            tc.tile_pool(name="osb", bufs=2) as osb,
        ):
            wt_t = per.tile([128, 64 * 128], dt)
            mt_t = per.tile([128, NCH * 8], F32)
            wy_t = per.tile([128, 8 * NCH], dt)
            by_t = per.tile([BLK, NCH], F32)
            hcarry = per.tile([128, 8], dt)
            hlast_sb = per.tile([128, 8], F32)
            nc.sync.dma_start(wt_t[:], wt[:])
            nc.sync.dma_start(mt_t[:], mt[:])
            nc.sync.dma_start(wy_t[:], wy[:])
            nc.sync.dma_start(by_t[:], by[:])
            nc.gpsimd.memset(hcarry[:], 0.0)

            psA = psumpool.tile([128, 8], F32, tag="psA")
            psB = psumpool.tile([128, 8], F32, tag="psB")
            ps = [psA, psB]
            hpreA = per.tile([128, 8], F32, tag="hpreA")
            hpreB = per.tile([128, 8], F32, tag="hpreB")
            hpre = [hpreA, hpreB]

            with tc.For_i(0, NBODY, name="scan") as b:
                idx_t = blkio.tile([1, BLK], I32, tag="idxblk")
                nc.sync.dma_start(idx_t[:], idx[0:1, bass.DynSlice(b * BLK, BLK)])
                hblk = blkio.tile([128, 8 * BLK], dt, tag="hblk")
                hblk_r = hblk[:].rearrange("p (k t) -> p k t", k=8)
                for s in range(BLK):
                    p = s % 2
                    for m in range(8):
                        for k in range(8):
                            rhs = (hcarry[:, k:k + 1] if s == 0
                                   else hblk_r[:, k, s - 1:s])
                            nc.tensor.matmul(
                                ps[p][:, m:m + 1],
                                wt_t[:, (m * 8 + k) * 128:(m * 8 + k) * 128 + 128],
                                rhs,
                                start=(k == 0),
                                stop=(k == 7),
                            )
                    r = nc.vector.alloc_register(f"idxr_{s}")
                    nc.vector.reg_load(r, idx_t[0:1, s:s + 1])
                    rv = nc.vector.snap(r, min_val=0, max_val=NCH - 1)
                    nc.vector.tensor_tensor(
                        hpre[p][:],
                        ps[p][:],
                        mt_t[:, bass.DynSlice(rv * 8, 8)],
                        op=mybir.AluOpType.add,
                    )
                    nc.scalar.activation(
                        hblk_r[:, :, s],
                        hpre[p][:],
                        mybir.ActivationFunctionType.Tanh,
                    )
                nc.sync.dma_start(
                    hst[:, bass.DynSlice(b * (8 * BLK), 8 * BLK)], hblk[:])
                nc.vector.tensor_copy(hcarry[:], hblk_r[:, :, BLK - 1])

            nc.scalar.activation(hlast_sb[:], hcarry[:],
                                 mybir.ActivationFunctionType.Copy)
            nc.sync.dma_start(hlast[:], hlast_sb[:])

            rb = nc.sync.alloc_register("myb_r")
            nc.sync.reg_load(rb, myb[0:1, 0:1].bitcast(U32))
            rbv = nc.sync.snap(rb, min_val=0, max_val=NBODY - NB_CORE)
            with tc.For_i(0, NB_CORE, name="oproj") as i:
                hst_t = blkio.tile([128, 8 * BLK], dt, tag="hstin")
                nc.sync.dma_start(
                    hst_t[:], hst[:, bass.DynSlice((rbv + i) * (8 * BLK), 8 * BLK)])
                ps2 = psumpool2.tile([BLK, NCH], F32, tag="ps2")
                for k in range(8):
                    nc.tensor.matmul(
                        ps2[:],
                        hst_t[:, k * BLK:(k + 1) * BLK],
                        wy_t[:, k * NCH:(k + 1) * NCH],
                        start=(k == 0),
                        stop=(k == 7),
                    )
                out_sb = osb.tile([BLK, NCH], F32, tag="outsb")
                nc.vector.tensor_tensor(
                    out_sb[:], ps2[:], by_t[:],
                    op=mybir.AluOpType.add,
                )
                nc.sync.dma_start(
                    out[bass.DynSlice(i * BLK, BLK), :], out_sb[:])

    nc.finalize()
    return nc


def _host_prep(input_seq, emb_table, Wax, Waa, Wya, b_y):
    np_dt = np.float16
    M = emb_table.astype(np.float32) @ Wax.astype(np.float32).T  # [512, 1024]
    mt = np.ascontiguousarray(
        M.reshape(NCH, 8, 128).transpose(2, 0, 1).reshape(128, NCH * 8)
    ).astype(np.float32)
    Wt = Waa.astype(np.float32).T
    wt = np.empty((128, 64 * 128), dtype=np_dt)
    for m in range(8):
        for k in range(8):
            wt[:, (m * 8 + k) * 128:(m * 8 + k) * 128 + 128] = \
                Wt[k * 128:(k + 1) * 128, m * 128:(m + 1) * 128].astype(np_dt)
    wy = np.ascontiguousarray(
        Wya.astype(np.float32).T.reshape(8, 128, NCH)
        .transpose(1, 0, 2).reshape(128, 8 * NCH)
    ).astype(np_dt)
    by = np.broadcast_to(
        b_y.astype(np.float32).reshape(1, NCH), (BLK, NCH)).copy()
    idxs = input_seq.astype(np.int32).reshape(1, SEQ)
    return wt, mt, wy, by, idxs


def _get_program():
    if "nc" not in _cache:
        _cache["nc"] = _build_program()
    return _cache["nc"]


def kernel(input_seq, emb_table, Wax, Waa, Wya, b_y):
    from concourse.bass_utils import run_bass_kernel_spmd

    input_seq = np.asarray(input_seq)
    in_dtype = input_seq.dtype
    wt, mt, wy, by, idxs = _host_prep(
        np.asarray(input_seq), np.asarray(emb_table), np.asarray(Wax),
        np.asarray(Waa), np.asarray(Wya), np.asarray(b_y))
    nc = _get_program()
    NB_CORE = NBODY // NCORES
    in_maps = [
        {"wt": wt, "mt": mt, "wy": wy, "by": by, "idx": idxs,
         "myb": np.array([[c * NB_CORE]], dtype=np.uint32)}
        for c in range(NCORES)
    ]
    res = run_bass_kernel_spmd(nc, in_maps, list(range(NCORES)))
    out = np.concatenate(
        [res.results[c]["out"] for c in range(NCORES)], axis=0)
    h_last = res.results[0]["hlast"].T.reshape(HID).astype(np.float32)
    return out, h_last


# revision 2
# speedup vs baseline: 1.5969x; 1.5969x over previous
"""CharRNN Trainium2 Bass kernel.

Problem: h_t = tanh(W_ax e_{x_t} + W_aa h_{t-1}); out = hs @ Wya^T + b_y.

Strategy:
  * Host folds embedding+input projection into one table  M = emb_table @ Wax^T
    (exact: row-gather commutes with the matmul), so on device the per-step
    input contribution is a dynamic column read of M^T (register-indexed AP).
  * The 65536-step scan is strictly sequential; every core runs the identical
    scan (replicated - cheapest correct option on this 8-core setup where
    cross-core SBUF p2p is unavailable and ncfw collectives have a ~5us/call
    floor).  The per-step matvec runs on the TensorEngine as 64 [128x128]
    weight-stationary fp16 tiles (fp32 PSUM accumulate); fp16 weights/state
    keep the final relative error ~2e-3 (the tanh dynamics are non-chaotic;
    errors saturate instead of growing).
  * Per step: PE 64 LDW+MM pairs -> DVE adds the (prefetched) x_t column from
    PSUM -> ACT tanh writes the fp16 h into the history block.  The dynamic
    x_t table read is prefetched one step ahead so the critical DVE op uses
    static APs.  BLK=8 steps per loop body measured fastest (~5.4us/step).
  * The output projection is S-sharded: each core projects its own 8192-step
    slice of the h history; the host concatenates the 8 shards.
"""
import sys
import numpy as np

_TRN_REPO = "/opt/trn_rl_repo"
if _TRN_REPO not in sys.path:
    sys.path.insert(0, _TRN_REPO)

SEQ = 65536
HID = 1024
EMB = 512
NCH = 512
NCORES = 8
BLK = 8           # scan steps per loop body
NBODY = SEQ // BLK

_cache = {}


def _build_program():
    import concourse.bacc as bacc
    import concourse.bass as bass
    import concourse.mybir as mybir
    from concourse import tile

    F32 = mybir.dt.float32
    F16 = mybir.dt.float16
    I32 = mybir.dt.int32
    U32 = mybir.dt.uint32
    dt = F16

    NB_CORE = NBODY // NCORES
    nc = bacc.Bacc(detect_race_conditions=False)

    wt = nc.declare_dram_parameter("wt", [128, 64 * 128], dt, isOutput=False)
    mt = nc.declare_dram_parameter("mt", [128, NCH * 8], F32, isOutput=False)
    wy = nc.declare_dram_parameter("wy", [128, 8 * NCH], dt, isOutput=False)
    by = nc.declare_dram_parameter("by", [BLK, NCH], F32, isOutput=False)
    idx = nc.declare_dram_parameter("idx", [1, SEQ], I32, isOutput=False)
    myb = nc.declare_dram_parameter("myb", [1, 1], U32, isOutput=False)
    out = nc.declare_dram_parameter("out", [NB_CORE * BLK, NCH], F32, isOutput=True)
    hlast = nc.declare_dram_parameter("hlast", [128, 8], F32, isOutput=True)
    hst = nc.dram_tensor("hst", [128, 8 * SEQ], dt)

    with tile.TileContext(nc) as tc:
        with (
            tc.tile_pool(name="per", bufs=1) as per,
            tc.tile_pool(name="blkio", bufs=2) as blkio,
            tc.tile_pool(name="psum", bufs=1, space="PSUM") as psumpool,
            tc.tile_pool(name="psum2", bufs=2, space="PSUM") as psumpool2,
            tc.tile_pool(name="osb", bufs=2) as osb,
        ):
            wt_t = per.tile([128, 64 * 128], dt)
            mt_t = per.tile([128, NCH * 8], F32)
            wy_t = per.tile([128, 8 * NCH], dt)
            by_t = per.tile([BLK, NCH], F32)
            hcarry = per.tile([128, 8], dt)
            hlast_sb = per.tile([128, 8], F32)
            nc.sync.dma_start(wt_t[:], wt[:])
            nc.sync.dma_start(mt_t[:], mt[:])
            nc.sync.dma_start(wy_t[:], wy[:])
            nc.sync.dma_start(by_t[:], by[:])
            nc.gpsimd.memset(hcarry[:], 0.0)

            psA = psumpool.tile([128, 8], F32, tag="psA")
            psB = psumpool.tile([128, 8], F32, tag="psB")
            ps = [psA, psB]
            hpreA = per.tile([128, 8], F32, tag="hpreA")
            hpreB = per.tile([128, 8], F32, tag="hpreB")
            hpre = [hpreA, hpreB]
            xptA = per.tile([128, 8], F32, tag="xptA")
            xptB = per.tile([128, 8], F32, tag="xptB")
            xpt = [xptA, xptB]

            with tc.For_i(0, NBODY, name="scan") as b:
                idx_t = blkio.tile([1, BLK], I32, tag="idxblk")
                nc.sync.dma_start(idx_t[:], idx[0:1, bass.DynSlice(b * BLK, BLK)])
                hblk = blkio.tile([128, 8 * BLK], dt, tag="hblk")
                hblk_r = hblk[:].rearrange("p (k t) -> p k t", k=8)
                for s in range(BLK):
                    p = s % 2
                    for m in range(8):
                        for k in range(8):
                            rhs = (hcarry[:, k:k + 1] if s == 0
                                   else hblk_r[:, k, s - 1:s])
                            nc.tensor.matmul(
                                ps[p][:, m:m + 1],
                                wt_t[:, (m * 8 + k) * 128:(m * 8 + k) * 128 + 128],
                                rhs,
                                start=(k == 0),
                                stop=(k == 7),
                            )
                    if s == 0:
                        # dynamic x_t column for step 0 of the body
                        r0 = nc.vector.alloc_register("idxr_a")
                        nc.vector.reg_load(r0, idx_t[0:1, 0:1])
                        rv0 = nc.vector.snap(r0, min_val=0, max_val=NCH - 1)
                        nc.vector.tensor_copy(
                            xpt[0][:], mt_t[:, bass.DynSlice(rv0 * 8, 8)])
                    nc.vector.tensor_tensor(
                        hpre[p][:], ps[p][:], xpt[p][:],
                        op=mybir.AluOpType.add,
                    )
                    nc.scalar.activation(
                        hblk_r[:, :, s], hpre[p][:],
                        mybir.ActivationFunctionType.Tanh,
                    )
                    if s + 1 < BLK:
                        # prefetch next step's x column off the critical path
                        rn = nc.vector.alloc_register(f"idxr_{s + 1}")
                        nc.vector.reg_load(rn, idx_t[0:1, s + 1:s + 2])
                        rvn = nc.vector.snap(rn, min_val=0, max_val=NCH - 1)
                        nc.vector.tensor_copy(
                            xpt[(s + 1) % 2][:],
                            mt_t[:, bass.DynSlice(rvn * 8, 8)])
                nc.sync.dma_start(
                    hst[:, bass.DynSlice(b * (8 * BLK), 8 * BLK)], hblk[:])
                nc.vector.tensor_copy(hcarry[:], hblk_r[:, :, BLK - 1])

            nc.scalar.activation(hlast_sb[:], hcarry[:],
                                 mybir.ActivationFunctionType.Copy)
            nc.sync.dma_start(hlast[:], hlast_sb[:])

            rb = nc.sync.alloc_register("myb_r")
            nc.sync.reg_load(rb, myb[0:1, 0:1].bitcast(U32))
            rbv = nc.sync.snap(rb, min_val=0, max_val=NBODY - NB_CORE)
            with tc.For_i(0, NB_CORE, name="oproj") as i:
                hst_t = blkio.tile([128, 8 * BLK], dt, tag="hstin")
                nc.sync.dma_start(
                    hst_t[:], hst[:, bass.DynSlice((rbv + i) * (8 * BLK), 8 * BLK)])
                ps2 = psumpool2.tile([BLK, NCH], F32, tag="ps2")
                for k in range(8):
                    nc.tensor.matmul(
                        ps2[:],
                        hst_t[:, k * BLK:(k + 1) * BLK],
                        wy_t[:, k * NCH:(k + 1) * NCH],
                        start=(k == 0),
                        stop=(k == 7),
                    )
                out_sb = osb.tile([BLK, NCH], F32, tag="outsb")
                nc.vector.tensor_tensor(
                    out_sb[:], ps2[:], by_t[:],
                    op=mybir.AluOpType.add,
                )
                nc.sync.dma_start(
                    out[bass.DynSlice(i * BLK, BLK), :], out_sb[:])

    nc.finalize()
    return nc


def _host_prep(input_seq, emb_table, Wax, Waa, Wya, b_y):
    np_dt = np.float16
    M = emb_table.astype(np.float32) @ Wax.astype(np.float32).T  # [512, 1024]
    mt = np.ascontiguousarray(
        M.reshape(NCH, 8, 128).transpose(2, 0, 1).reshape(128, NCH * 8)
    ).astype(np.float32)
    Wt = Waa.astype(np.float32).T
    wt = np.empty((128, 64 * 128), dtype=np_dt)
    for m in range(8):
        for k in range(8):
            wt[:, (m * 8 + k) * 128:(m * 8 + k) * 128 + 128] = \
                Wt[k * 128:(k + 1) * 128, m * 128:(m + 1) * 128].astype(np_dt)
    wy = np.ascontiguousarray(
        Wya.astype(np.float32).T.reshape(8, 128, NCH)
        .transpose(1, 0, 2).reshape(128, 8 * NCH)
    ).astype(np_dt)
    by = np.broadcast_to(
        b_y.astype(np.float32).reshape(1, NCH), (BLK, NCH)).copy()
    idxs = input_seq.astype(np.int32).reshape(1, SEQ)
    return wt, mt, wy, by, idxs


def _get_program():
    if "nc" not in _cache:
        _cache["nc"] = _build_program()
    return _cache["nc"]


def kernel(input_seq, emb_table, Wax, Waa, Wya, b_y):
    from concourse.bass_utils import run_bass_kernel_spmd

    wt, mt, wy, by, idxs = _host_prep(
        np.asarray(input_seq), np.asarray(emb_table), np.asarray(Wax),
        np.asarray(Waa), np.asarray(Wya), np.asarray(b_y))
    nc = _get_program()
    NB_CORE = NBODY // NCORES
    in_maps = [
        {"wt": wt, "mt": mt, "wy": wy, "by": by, "idx": idxs,
         "myb": np.array([[c * NB_CORE]], dtype=np.uint32)}
        for c in range(NCORES)
    ]
    res = run_bass_kernel_spmd(nc, in_maps, list(range(NCORES)))
    out = np.concatenate(
        [res.results[c]["out"] for c in range(NCORES)], axis=0)
    h_last = res.results[0]["hlast"].T.reshape(HID).astype(np.float32)
    return out, h_last
